# revision 2
# baseline (speedup 1.0000x reference)
"""Trainium2 Bass kernel for nn_Eq1to3 (gnn_message_passing).

Reference computation:
    Y  = einsum('ndi,dsb->nsbi', x, coefs[:, :, :3])      # (n, s, 3, m)
    S  = einsum('nd,ds->ns', x.sum(-1), coefs[:, :, 3])   # (n, s)
    out[n,s,i,j,k] = Y0[n,s,i] + Y1[n,s,j] + Y2[n,s,k] + S[n,s] + bias[s]

Shapes: x (4, 16, 96) f32 -> out (4, 16, 96, 96, 96) f32 (~226.5 MB).
The contractions are microscopic (a few MFLOP); the real work is
materializing and writing 226 MB — the kernel is HBM-write bound.

Strategy (8 NeuronCores):
  * Shard (n, i): core c handles n = c//2, i in [48*(c%2), 48*(c%2)+48).
    Per-core output 28.3 MB — perfectly balanced, no collectives.
  * Host precomputes (microscopic contractions, fp32 exact):
        W[n, s, (j,k)] = Y1[n,s,j] + Y2[n,s,k] + S[n,s] + bias[s]   (i-free)
        A[n, s, i]     = Y0[n,s,i]
    and ships ONE input tensor per core, winp [128, 9224]:
        cols 0:9216   = W0 = W replicated over the 8 i'-rows of each s-group,
                        with chunk 0's A column pre-added (a_0 folded in),
        cols 9216:9222 = delta columns d_t = a_t - a_0  (d_0 = 0, unused).
  * Device tile layout: 128 partitions = (s: 16) x (i'-in-chunk: 8), free
    dim = (j,k) = 9216.  Six i-chunks of 8 cover the 48 i-values.
  * Device body (13 instructions):
        load d columns (tiny), load W0 as two half DMAs (sync+scalar rings),
        write chunk 0 STRAIGHT from the W0 tile (no compute),
        chunks 1..5: one DVE tensor_scalar_add each (big = W0 + d_t),
        write 4.72 MB per chunk, alternating the two HWDGE rings (SP/ACT).
  * Per-core HBM traffic = 28.3 MB written + 4.7 MB read -> 92 us roofline
    at ~358 GB/s per-core HBM bandwidth; measured ~100 us per execution
    (R-replicated steady-state differencing, which cancels dispatch).
    All DVE compute (~33 us) is hidden behind the DMA writes.
    fp32 end to end: rel err vs fp32 reference ~1e-6.

The per-core output layout is chunk-major (t, s, i', j*96+k) so every DMA
destination is contiguous; the host gathers/permutes shards into the full
(4, 16, 96, 96, 96) array.

(SWDGE/gpsimd output DMAs are deliberately avoided: in an earlier session
they correlated with rare NRT_EXEC_UNIT_UNRECOVERABLE device crashes.)
"""

import sys

sys.path.insert(0, "/opt/trn_rl_repo")

import numpy as np

import concourse.bacc as bacc
import concourse.mybir as mybir
from concourse.tile import TileContext
from concourse.bass_utils import run_bass_kernel_spmd

N_BATCH = 4
IN_DIM = 16
OUT_DIM = 16
M = 96
JK = M * M  # 9216
N_CORES = 8
I_PER_CORE = 48  # one n, half of the i axis per core
I_CHUNK = 8  # 16 s * 8 i' = 128 partitions
N_CHUNKS = I_PER_CORE // I_CHUNK  # 6
WCOLS = JK + 8  # replicated W0 + 6 delta columns + 2 pad

_PROGRAM_CACHE = {}


def _build_program(R=1):
    """Build the per-core program; R > 1 replicates the body R times for
    steady-state device-time measurement (test harness use only)."""
    nc = bacc.Bacc(None)
    w_d = nc.dram_tensor("w", [128, WCOLS], mybir.dt.float32, kind="ExternalInput")
    o_d = nc.dram_tensor(
        "o", [N_CHUNKS, OUT_DIM, I_CHUNK, JK], mybir.dt.float32, kind="ExternalOutput"
    )
    half = JK // 2
    with TileContext(nc) as tc:
        with (
            tc.tile_pool(name="spool", bufs=1) as spool,
            tc.tile_pool(name="b0pool", bufs=1) as b0pool,
            tc.tile_pool(name="bigpool", bufs=4) as bigpool,
        ):
            d_sb = spool.tile([128, 8], mybir.dt.float32)
            nc.scalar.dma_start(out=d_sb[:], in_=w_d[:, JK : JK + 8])
            engs = [nc.sync, nc.scalar]
            for _ in range(R):
                big0 = b0pool.tile([128, JK], mybir.dt.float32)
                nc.sync.dma_start(out=big0[:, :half], in_=w_d[:, :half])
                nc.scalar.dma_start(out=big0[:, half:], in_=w_d[:, half:JK])
                for t in range(N_CHUNKS):
                    if t == 0:
                        src = big0  # a_0 is folded into W0 on the host
                    else:
                        src = bigpool.tile([128, JK], mybir.dt.float32)
                        nc.vector.tensor_scalar_add(
                            out=src[:], in0=big0[:], scalar1=d_sb[:, t : t + 1]
                        )
                    engs[t % 2].dma_start(out=o_d[t], in_=src[:])
    nc.compile()
    return nc


def _host_precompute(x, coefs, bias):
    x = np.asarray(x, dtype=np.float32)
    coefs = np.asarray(coefs, dtype=np.float32)
    bias = np.asarray(bias, dtype=np.float32)
    Y = np.einsum("ndi,dsb->nsbi", x, coefs[:, :, :3], optimize=True).astype(np.float32)
    S = np.einsum("nd,ds->ns", x.sum(axis=-1), coefs[:, :, 3], optimize=True).astype(
        np.float32
    )
    A = Y[:, :, 0, :]  # (n, s, i)
    Y1 = Y[:, :, 1, :]  # (n, s, j)
    Z2 = Y[:, :, 2, :] + (S + bias.reshape(1, OUT_DIM))[:, :, None]  # (n, s, k)
    W = (Y1[:, :, :, None] + Z2[:, :, None, :]).reshape(N_BATCH, OUT_DIM, JK)
    return W.astype(np.float32), A.astype(np.float32)


def _make_in_maps(W, A):
    in_maps = []
    for c in range(N_CORES):
        n = c // 2
        i0 = (c % 2) * I_PER_CORE
        # a_in[p=(s,i'), t] = A[n, s, i0 + t*8 + i']
        a_in = (
            A[n, :, i0 : i0 + I_PER_CORE]
            .reshape(OUT_DIM, N_CHUNKS, I_CHUNK)
            .transpose(0, 2, 1)
            .reshape(128, N_CHUNKS)
        )
        wrep = np.repeat(W[n].reshape(OUT_DIM, 1, JK), I_CHUNK, axis=1).reshape(
            128, JK
        )
        winp = np.zeros((128, WCOLS), np.float32)
        winp[:, :JK] = wrep + a_in[:, 0:1]
        winp[:, JK : JK + N_CHUNKS] = a_in - a_in[:, 0:1]
        in_maps.append({"w": np.ascontiguousarray(winp)})
    return in_maps


def _run(inputs, trace=False, **kwargs):
    W, A = _host_precompute(inputs["x"], inputs["coefs"], inputs["bias"])
    if "nc" not in _PROGRAM_CACHE:
        _PROGRAM_CACHE["nc"] = _build_program()
    nc = _PROGRAM_CACHE["nc"]
    in_maps = _make_in_maps(W, A)
    res = run_bass_kernel_spmd(nc, in_maps, list(range(N_CORES)), trace=trace, **kwargs)

    out = np.empty((N_BATCH, OUT_DIM, M, M, M), dtype=np.float32)
    for c in range(N_CORES):
        n = c // 2
        i0 = (c % 2) * I_PER_CORE
        blk = res.results[c]["o"].reshape(N_CHUNKS, OUT_DIM, I_CHUNK, M, M)
        out[n, :, i0 : i0 + I_PER_CORE] = blk.transpose(1, 0, 2, 3, 4).reshape(
            OUT_DIM, I_PER_CORE, M, M
        )
    return out, res


def kernel(**inputs) -> np.ndarray:
    out, _ = _run(inputs, trace=False)
    return out


if __name__ == "__main__":
    rng = np.random.default_rng(0)
    x = rng.standard_normal((N_BATCH, IN_DIM, M), dtype=np.float32)
    coefs = rng.standard_normal((IN_DIM, OUT_DIM, 4), dtype=np.float32)
    bias = np.zeros((1, OUT_DIM, 1, 1, 1), dtype=np.float32)
    out = kernel(x=x, coefs=coefs, bias=bias)
    Y = np.einsum("ndi,dsb->nsbi", x, coefs[:, :, :3])
    S = np.einsum("nd,ds->ns", x.sum(-1), coefs[:, :, 3])
    exp = (
        Y[:, :, 0, :, None, None]
        + Y[:, :, 1, None, :, None]
        + Y[:, :, 2, None, None, :]
        + S[:, :, None, None, None]
    )
    print("smoke max err:", float(np.abs(out - exp).max()))


# revision 6
# speedup vs baseline: 1.4795x; 1.4795x over previous
"""Trainium2 Bass kernel for nn_Eq1to3 (gnn_message_passing).

Reference computation:
    Y  = einsum('ndi,dsb->nsbi', x, coefs[:, :, :3])      # (n, s, 3, m)
    S  = einsum('nd,ds->ns', x.sum(-1), coefs[:, :, 3])   # (n, s)
    out[n,s,i,j,k] = Y0[n,s,i] + Y1[n,s,j] + Y2[n,s,k] + S[n,s] + bias[s]

Shapes: x (4, 16, 96) f32 -> out (4, 16, 96, 96, 96) f32 (~226.5 MB).
The contractions are microscopic (a few MFLOP); the real work is
materializing and writing 226 MB — the kernel is HBM-write bound.

Strategy (8 NeuronCores):
  * Shard (n, i): core c handles n = c//2, i in [48*(c%2), 48*(c%2)+48).
    Per-core output 28.3 MB — perfectly balanced, no collectives.
  * Host precomputes only the tiny contractions and ships ~106 KB per core
    (ss [128, 208] f32): per partition p=(s,i') the rows
        y1 = Y1[n,s,:]  (96),   z2 = Y2[n,s,:] + S[n,s] + bias[s]  (96),
        a_t = Y0[n, s, i0 + t*8 + i']  (6 columns).
  * The 226 MB tensor is BUILT ON CHIP: out slab = y1[j] + z2[k] + a_t —
    an outer sum — computed by ONE fused DVE `scalar_tensor_tensor` op per
    (chunk t, piece q):  out = (z2_bcast + a_t) + y1_bcast, where the
    broadcasts are zero-stride access patterns (z2 row re-read for each of
    24 j-values; each y1 element repeated 96x).  fp32-exact end to end.
  * Device tile layout: 128 partitions = (s: 16) x (i'-in-chunk: 8), free
    dim = (j,k).  6 i-chunks x 4 column pieces = 24 fused DVE ops (~41 us,
    hidden) + 24 output DMAs of 1.18 MB alternating the two HWDGE rings.
  * Per-core HBM traffic = 28.3 MB written + 0.1 MB read.  Measured
    ~78 us per execution vs ~73 us for a pure-write kernel of the same
    output (R-replicated paired differencing, which cancels the axon
    dispatch overhead) — i.e. ~390 GB/s effective per-core write bandwidth,
    with the on-chip build ~95% hidden behind the writes.

The per-core output layout is chunk-major (t, s, i', j*96+k); the host
gathers/permutes shards into the full (4, 16, 96, 96, 96) array.

(SWDGE/gpsimd output DMAs are deliberately avoided: in an earlier session
they correlated with rare NRT_EXEC_UNIT_UNRECOVERABLE device crashes.)
"""

import dataclasses
import sys

sys.path.insert(0, "/opt/trn_rl_repo")

import numpy as np

import concourse.bacc as bacc
import concourse.mybir as mybir
from concourse.tile import TileContext
from concourse.bass_utils import run_bass_kernel_spmd

N_BATCH = 4
IN_DIM = 16
OUT_DIM = 16
M = 96
JK = M * M  # 9216
N_CORES = 8
I_PER_CORE = 48  # one n, half of the i axis per core
I_CHUNK = 8  # 16 s * 8 i' = 128 partitions
N_CHUNKS = I_PER_CORE // I_CHUNK  # 6
Q_PIECES = 4  # column pieces per chunk (24 j-values x 96 k each)
SCOLS = 2 * M + 16  # ss row: y1 | z2 | a columns | pad

_PROGRAM_CACHE = {}


def _build_program(R=1):
    """Build the per-core program; R > 1 replicates the body R times for
    steady-state device-time measurement (test harness use only)."""
    nc = bacc.Bacc(None)
    s_d = nc.dram_tensor("ss", [128, SCOLS], mybir.dt.float32, kind="ExternalInput")
    o_d = nc.dram_tensor(
        "o", [N_CHUNKS, OUT_DIM, I_CHUNK, JK], mybir.dt.float32, kind="ExternalOutput"
    )
    jq = M // Q_PIECES  # j-values per piece (24)
    pcols = jq * M  # columns per piece (2304)
    with TileContext(nc) as tc:
        with (
            tc.tile_pool(name="spool", bufs=1) as spool,
            tc.tile_pool(name="piecepool", bufs=8) as piecepool,
        ):
            ss = spool.tile([128, SCOLS], mybir.dt.float32)
            nc.scalar.dma_start(out=ss[:], in_=s_d[:])
            pstride = ss[:].ap[0][0]
            engs = [nc.sync, nc.scalar]
            ring = 0
            for _ in range(R):
                for q in range(Q_PIECES):
                    sl = slice(q * pcols, (q + 1) * pcols)
                    for t in range(N_CHUNKS):
                        piece = piecepool.tile([128, pcols], mybir.dt.float32)
                        # z2 row re-read for each of the jq j-values
                        z2 = dataclasses.replace(
                            ss[:, M : 2 * M],
                            ap=[[pstride, 128], [0, jq], [1, M]],
                        )
                        # y1 element repeated M times (k-axis broadcast)
                        y1 = dataclasses.replace(
                            ss[:, q * jq : (q + 1) * jq],
                            ap=[[pstride, 128], [1, jq], [0, M]],
                        )
                        outap = dataclasses.replace(
                            piece[:],
                            ap=[[piece[:].ap[0][0], 128], [M, jq], [1, M]],
                        )
                        a_t = ss[:, 2 * M + t : 2 * M + t + 1]
                        nc.vector.scalar_tensor_tensor(
                            out=outap,
                            in0=z2,
                            scalar=a_t,
                            in1=y1,
                            op0=mybir.AluOpType.add,
                            op1=mybir.AluOpType.add,
                        )
                        engs[ring].dma_start(out=o_d[t][:, :, sl], in_=piece[:])
                        ring ^= 1
    nc.compile()
    return nc


def _host_precompute(x, coefs, bias):
    x = np.asarray(x, dtype=np.float32)
    coefs = np.asarray(coefs, dtype=np.float32)
    bias = np.asarray(bias, dtype=np.float32)
    Y = np.einsum("ndi,dsb->nsbi", x, coefs[:, :, :3], optimize=True).astype(np.float32)
    S = np.einsum("nd,ds->ns", x.sum(axis=-1), coefs[:, :, 3], optimize=True).astype(
        np.float32
    )
    A = Y[:, :, 0, :]  # (n, s, i)
    Y1 = Y[:, :, 1, :]  # (n, s, j)
    Z2 = Y[:, :, 2, :] + (S + bias.reshape(1, OUT_DIM))[:, :, None]  # (n, s, k)
    return Y1, Z2, A


def _make_in_maps(Y1, Z2, A):
    in_maps = []
    for c in range(N_CORES):
        n = c // 2
        i0 = (c % 2) * I_PER_CORE
        # a_in[p=(s,i'), t] = A[n, s, i0 + t*8 + i']
        a_in = (
            A[n, :, i0 : i0 + I_PER_CORE]
            .reshape(OUT_DIM, N_CHUNKS, I_CHUNK)
            .transpose(0, 2, 1)
            .reshape(128, N_CHUNKS)
        )
        ss = np.zeros((128, SCOLS), np.float32)
        ss[:, :M] = np.repeat(Y1[n], I_CHUNK, axis=0)
        ss[:, M : 2 * M] = np.repeat(Z2[n], I_CHUNK, axis=0)
        ss[:, 2 * M : 2 * M + N_CHUNKS] = a_in
        in_maps.append({"ss": np.ascontiguousarray(ss)})
    return in_maps


def _run(inputs, trace=False, **kwargs):
    import time as _time

    Y1, Z2, A = _host_precompute(inputs["x"], inputs["coefs"], inputs["bias"])
    if "nc" not in _PROGRAM_CACHE:
        _PROGRAM_CACHE["nc"] = _build_program()
    nc = _PROGRAM_CACHE["nc"]
    in_maps = _make_in_maps(Y1, Z2, A)
    # The axon terminal occasionally reports a transient
    # NRT_EXEC_UNIT_UNRECOVERABLE right after another process exits; it
    # clears within seconds, so retry with backoff before giving up.
    last_err = None
    for attempt in range(3):
        try:
            res = run_bass_kernel_spmd(
                nc, in_maps, list(range(N_CORES)), trace=trace, **kwargs
            )
            break
        except (ImportError, TypeError, ValueError, AssertionError):
            raise  # non-transient
        except Exception as e:  # noqa: BLE001
            last_err = e
            _time.sleep(5 * (attempt + 1))
    else:
        raise last_err

    out = np.empty((N_BATCH, OUT_DIM, M, M, M), dtype=np.float32)
    for c in range(N_CORES):
        n = c // 2
        i0 = (c % 2) * I_PER_CORE
        blk = res.results[c]["o"].reshape(N_CHUNKS, OUT_DIM, I_CHUNK, M, M)
        out[n, :, i0 : i0 + I_PER_CORE] = blk.transpose(1, 0, 2, 3, 4).reshape(
            OUT_DIM, I_PER_CORE, M, M
        )
    return out, res


def kernel(**inputs) -> np.ndarray:
    out, _ = _run(inputs, trace=False)
    return out


if __name__ == "__main__":
    rng = np.random.default_rng(0)
    x = rng.standard_normal((N_BATCH, IN_DIM, M), dtype=np.float32)
    coefs = rng.standard_normal((IN_DIM, OUT_DIM, 4), dtype=np.float32)
    bias = np.zeros((1, OUT_DIM, 1, 1, 1), dtype=np.float32)
    out = kernel(x=x, coefs=coefs, bias=bias)
    Y = np.einsum("ndi,dsb->nsbi", x, coefs[:, :, :3])
    S = np.einsum("nd,ds->ns", x.sum(-1), coefs[:, :, 3])
    exp = (
        Y[:, :, 0, :, None, None]
        + Y[:, :, 1, None, :, None]
        + Y[:, :, 2, None, None, :]
        + S[:, :, None, None, None]
    )
    print("smoke max err:", float(np.abs(out - exp).max()))
